# revision 25
# baseline (speedup 1.0000x reference)
"""GCN layer (gather -> x@W -> normalized scatter-add -> bias -> PReLU) on 8 trn2 cores.

Strategy (no collectives; x replicated, full hs table computed locally per core):
  - 100000 nodes padded to 102400 = 8 * 12800; core c owns dst nodes [c*12800, (c+1)*12800).
  - Per-core rotated table layout: on core c, table slot t holds node
    (t + c*12800) % 102400 — realized purely via each core's host-built x input
    permutation. Every core's own dst shard is then table slots [0, 12800),
    shared compile-time positions, and real cross-partition edges hit quarters
    uniformly, keeping the shared schedule's max-over-cores padding small.
  - Host: x' = dinv[:,None] * x (folds the src-side norm), padded, cast bf16,
    K-major [256, 102400], columns permuted so phase-1 matmul chunk (g, c)
    yields, at out partition p, table slot g*512 + p*4 + c.
  - Phase 1: per 512-slot group: 8 matmuls (k=256 split in 2) into PSUM
    [128, 4*128], Activation copy to bf16 SBUF; x loads and table writes are
    batched 4 groups per DMA with 1KB+ contiguous elements. Table stored as 4
    quarter tensors [25600, 128] bf16 so gather idx fit int16. Groups 0-24
    (own dst shard) are kept resident in SBUF (hs_own) as well.
  - Phase 3: self-loop contributions come from hs_own via 4 constant
    selection-matrix matmuls per window (no gather rows spent on them).
    Real edges sorted by (dst group gg of 8 windows, src quarter q, dst
    window); per (gg,q,w) segment length = max count over cores (edge
    granularity), each (gg,q) bin padded to 128 once, so 128-edge blocks may
    straddle one window boundary. Per block a one-hot S[e, d] =
    (iota_bank == rel[e]) is built on DVE (rel is dst offset from the block's
    first window, 0..255, bf16-exact; bank j covers 128j..128j+127); PE
    accumulates psum[d, f] += S^T @ G per overlapped window, G = dma_gather'ed
    hs rows (idx streams stored 16-partition, not replicated). Epilogue:
    out = prelu(dinv_dst * psum + b) as max(u, alpha*u) when b == 0 and alpha
    uniform in [0,1] (true here); bf16 output written partition-major
    [128, NW*H], upcast + transposed back on host.
"""
import sys
sys.path.insert(0, '/opt/trn_rl_repo')

import numpy as np
import ml_dtypes

N = 100000
NCORES = 8
SH = 12800                 # dst nodes per core
NP = NCORES * SH           # 102400 padded nodes
H = 128                    # output features
KIN = 256                  # input features
WIN = 128                  # dst window size
NW = SH // WIN             # 100 windows per core
WG = 8                     # windows per PSUM group
NG = (NW + WG - 1) // WG   # 13 groups (last has 4)
NQ = 4                     # source quarters (int16 gather idx: 25600 < 32768)
QTAB = NP // NQ            # 25600 rows per quarter table
GRP = 512                  # phase-1 rows per PSUM group
NGRP = NP // GRP           # 200 phase-1 groups
XB = 4                     # phase-1 groups per DMA batch
NOWN = SH // GRP           # 25 groups resident in SBUF (own dst shard)
PREF = 4                   # idx loads prefetched ahead of phase 3

bf16 = ml_dtypes.bfloat16


def _preprocess(edge_index):
    e_src = np.asarray(edge_index[0]).astype(np.int64)
    e_dst = np.asarray(edge_index[1]).astype(np.int64)

    deg = (np.bincount(e_dst, minlength=N) + 1).astype(np.float32)
    dinv = (1.0 / np.sqrt(deg)).astype(np.float32)
    dinv_np = np.ones(NP, np.float32)
    dinv_np[:N] = dinv

    # real edges only; self-loops are applied from SBUF in phase 3
    src = e_src
    dst = e_dst
    E = src.shape[0]

    core = dst // SH
    w_in_core = (dst % SH) // WIN            # 0..NW-1
    g = w_in_core // WG
    wi = w_in_core % WG
    rot = (src - core * SH) % NP             # per-core rotated table slot
    q = rot // QTAB                          # source quarter
    tab_row = rot % QTAB                     # row within quarter table

    # sort by (core, g, q, window)
    key = ((core * NG + g) * NQ + q) * WG + wi
    nbins_pc = NG * NQ * WG
    order = np.argsort(key, kind='stable')
    o_tab = tab_row[order]
    o_dst = dst[order]
    o_key = key[order]
    o_core = core[order]

    cnt_all = np.bincount(key, minlength=NCORES * nbins_pc)
    bin_start = np.concatenate([[0], np.cumsum(cnt_all)])[:-1]
    rank = np.arange(E, dtype=np.int64) - bin_start[o_key]   # within (c,g,q,w)

    # every window must have at least one real edge on SOME core, so the
    # shared schedule has a stop-flag anchor for each PSUM region
    wcnt = cnt_all.reshape(NCORES, NG, NQ, WG).sum(axis=(0, 2)).reshape(-1)
    assert wcnt[:NW].min() >= 1

    # shared schedule: per (g,q,w) segment length = max count over cores
    # (edge granularity); each (g,q) bin padded to a 128 multiple once.
    maxcnt = cnt_all.reshape(NCORES, NG, NQ, WG).max(axis=0)  # [NG, NQ, WG]
    seg_end = np.cumsum(maxcnt, axis=2)
    wstart = seg_end - maxcnt                                 # [NG, NQ, WG]
    binlen = ((seg_end[:, :, -1] + WIN - 1) // WIN) * WIN     # [NG, NQ]
    offs_gq = np.concatenate([[0], np.cumsum(binlen.reshape(-1))])  # per (g,q)
    TOT = int(offs_gq[-1])
    NBLK = TOT // WIN

    gqw_in_core = o_key % nbins_pc
    gq_in_core = gqw_in_core // WG
    pos = (offs_gq[gq_in_core] + wstart.reshape(-1)[gqw_in_core] + rank)
    blkid = pos // WIN                                        # global block

    # per-block overlapped windows from the shared segment layout
    wmin_blk = np.zeros(NBLK, np.int64)
    blk_wins = [[] for _ in range(NBLK)]
    for gg in range(NG):
        for qq in range(NQ):
            bin_i = gg * NQ + qq
            b0 = int(offs_gq[bin_i]) // WIN
            nb = int(binlen[gg, qq]) // WIN
            for wii in range(WG):
                w = gg * WG + wii
                if w >= NW or maxcnt[gg, qq, wii] == 0:
                    continue
                lo = int(wstart[gg, qq, wii])
                hi = int(seg_end[gg, qq, wii])
                for b in range(b0 + lo // WIN, b0 + (hi - 1) // WIN + 1):
                    blk_wins[b].append(w)
            for b in range(b0, b0 + nb):
                assert len(blk_wins[b]) <= 2, "block spans >2 windows"
                wmin_blk[b] = blk_wins[b][0] if blk_wins[b] else 0

    rel = (o_dst - (o_core * SH + wmin_blk[blkid] * WIN)).astype(np.float32)
    assert rel.min() >= 0 and rel.max() <= 255.0

    per_core = []
    for c in range(NCORES):
        m = o_core == c
        idxq = np.zeros(TOT, np.int16)
        rels = np.full(TOT, -1.0, np.float32)
        p_c = pos[m]
        idxq[p_c] = o_tab[m].astype(np.int16)
        rels[p_c] = rel[m]
        idx16 = np.tile(np.ascontiguousarray(idxq.reshape(TOT // 16, 16).T), (8, 1))
        relm = np.ascontiguousarray(rels.reshape(NBLK, WIN).T)   # [128, NBLK]
        dinv_own = np.ascontiguousarray(
            dinv_np[c * SH:(c + 1) * SH].reshape(NW, WIN).T)     # [128, NW]
        per_core.append(dict(idx16=idx16, reldst=relm, dinv=dinv_own))

    # build calls: per (g, q): gather binlen[g,q] rows; per block the
    # matmul list [(Bcol, w, bank)] for each overlapped window
    calls = []          # (g, q, off_idx, nidx, [(Bcol, w, bank), ...])
    last_block_of_win = {}
    for gg in range(NG):
        for qq in range(NQ):
            nlen = int(binlen[gg, qq])
            if nlen == 0:
                continue
            bin_i = gg * NQ + qq
            off_idx = int(offs_gq[bin_i])
            b0 = off_idx // WIN
            mms = []
            for b in range(b0, b0 + nlen // WIN):
                for w in blk_wins[b]:
                    mms.append((b, w, w - blk_wins[b][0]))
                    last_block_of_win[w] = (b, w)
            calls.append((gg, qq, off_idx, nlen, mms))
    sched = dict(calls=calls, last_block=last_block_of_win,
                 NBLK=NBLK, TOT=TOT)
    return sched, per_core, dinv_np


def _build(sched, fast_epilogue=None, self_mode="sel"):
    from concourse import bass, bacc, tile, mybir

    nc = bacc.Bacc("TRN2", target_bir_lowering=False, debug=False,
                   enable_asserts=True, num_devices=NCORES)

    xt_d = nc.dram_tensor("xt_perm", [KIN, NP], mybir.dt.bfloat16, kind="ExternalInput")
    w_d = nc.dram_tensor("w_bf", [KIN, H], mybir.dt.bfloat16, kind="ExternalInput")
    b_d = nc.dram_tensor("b_vec", [H], mybir.dt.float32, kind="ExternalInput")
    a_d = nc.dram_tensor("a_vec", [H], mybir.dt.float32, kind="ExternalInput")
    dinv_d = nc.dram_tensor("dinv_own", [128, NW], mybir.dt.float32, kind="ExternalInput")
    idx_d = nc.dram_tensor("idx16", [128, sched["TOT"] // 16], mybir.dt.int16, kind="ExternalInput")
    rel_d = nc.dram_tensor("reldst", [128, sched["NBLK"]], mybir.dt.float32, kind="ExternalInput")

    # output in partition-major layout: out[d, w*H + f] = result[w*128 + d, f]
    out_d = nc.dram_tensor("out_pm", [128, NW * H], mybir.dt.bfloat16, kind="ExternalOutput")

    hs_tab = [nc.dram_tensor(f"hs_tab{k}", [QTAB, H], mybir.dt.bfloat16) for k in range(NQ)]

    calls = sched["calls"]
    last_block = sched["last_block"]
    first_block = {}
    for (_gg, _qq, _oi, _ni, _mms) in calls:
        for (b, w, _bank) in _mms:
            first_block.setdefault(w, (b, w))
    max_call_blk = max(cb[3] // WIN for cb in calls)
    nblk_of_group = [sum(cb[3] // WIN for cb in calls if cb[0] == gg) for gg in range(NG)]
    first_col_of_group = [min([cb[2] // WIN for cb in calls if cb[0] == gg] or [0])
                          for gg in range(NG)]

    with tile.TileContext(nc) as tc:
        with tc.tile_pool(name="consts", bufs=1) as cp, tc.tile_pool(name="sb", bufs=3) as sb:
            # ---------------- constants ----------------
            iota_bank = []
            for j in range(2):
                it = cp.tile([128, 128], mybir.dt.int32, tag=f"it{j}")
                nc.gpsimd.iota(it[:], pattern=[[1, 128]], base=j * 128,
                               channel_multiplier=0)
                ib = cp.tile([128, 128], mybir.dt.bfloat16, tag=f"ib{j}")
                nc.vector.tensor_copy(ib[:], it[:])
                iota_bank.append(ib)

            # selection matrices for self-loop rows: sel[r][c][p, d] = 1 iff
            # d == 4p - 128r + c  (window w = 4*g_own + r, chunk c)
            selmat = []
            for r in range(4):
                row = []
                for c in range(4):
                    cv = cp.tile([128, 1], mybir.dt.int32, tag=f"cv{r}{c}")
                    nc.gpsimd.iota(cv[:], pattern=[[0, 1]], base=c - 128 * r,
                                   channel_multiplier=4)
                    cvf = cp.tile([128, 1], mybir.dt.float32, tag=f"cvf{r}{c}")
                    nc.vector.tensor_copy(cvf[:], cv[:])
                    sm = cp.tile([128, 128], mybir.dt.bfloat16, tag=f"sm{r}{c}")
                    nc.vector.tensor_scalar(
                        out=sm[:], in0=iota_bank[0][:],
                        scalar1=cvf[:], scalar2=None,
                        op0=mybir.AluOpType.is_equal)
                    row.append(sm)
                selmat.append(row)

            w0 = cp.tile([128, H], mybir.dt.bfloat16)
            w1 = cp.tile([128, H], mybir.dt.bfloat16)
            nc.sync.dma_start(w0[:], w_d[0:128, :])
            nc.sync.dma_start(w1[:], w_d[128:256, :])

            dinv_sb = cp.tile([128, NW], mybir.dt.float32)
            nc.sync.dma_start(dinv_sb[:], dinv_d[:, :])

            ones1 = cp.tile([1, H], mybir.dt.float32)
            nc.vector.memset(ones1[:], 1.0)
            bvec = cp.tile([1, H], mybir.dt.float32)
            nc.sync.dma_start(bvec[:], b_d[None, :])
            avec = cp.tile([1, H], mybir.dt.float32)
            nc.sync.dma_start(avec[:], a_d[None, :])

            b128 = cp.tile([128, H], mybir.dt.float32)
            a128 = cp.tile([128, H], mybir.dt.float32)

            hs_own = cp.tile([128, NOWN, GRP], mybir.dt.bfloat16)  # 25KB/part

            nidx_cols = sched["TOT"] // 16
            idx_all = cp.tile([128, nidx_cols], mybir.dt.int16)   # ~28KB/part
            half = (nidx_cols // 2) & ~7
            nc.sync.dma_start(idx_all[:, 0:half], idx_d[:, 0:half])
            nc.sync.dma_start(idx_all[:, half:nidx_cols], idx_d[:, half:nidx_cols])

            with tc.tile_pool(name="psum1", bufs=1, space="PSUM") as pp1:
                if not fast_epilogue:
                    bc_ps = pp1.tile([128, H], mybir.dt.float32, space="PSUM", tag="bc", bufs=1)
                    nc.tensor.matmul(out=bc_ps[:], lhsT=ones1[:], rhs=bvec[:], start=True, stop=True)
                    nc.vector.tensor_copy(b128[:], bc_ps[:])
                    ac_ps = pp1.tile([128, H], mybir.dt.float32, space="PSUM", tag="bc", bufs=1)
                    nc.tensor.matmul(out=ac_ps[:], lhsT=ones1[:], rhs=avec[:], start=True, stop=True)
                    nc.vector.tensor_copy(a128[:], ac_ps[:])

                # ---------------- phase 1: full hs table, local ----------------
                for bb in range(NGRP // XB):
                    g0 = bb * XB
                    x_t = sb.tile([128, 2, XB * GRP], mybir.dt.bfloat16, tag="x_t", bufs=3)
                    nc.sync.dma_start(
                        x_t[:],
                        xt_d[:, g0 * GRP:(g0 + XB) * GRP].rearrange(
                            "(a p) c -> p a c", p=128))
                    hb = sb.tile([128, XB, 4 * H], mybir.dt.bfloat16, tag="hb", bufs=3)
                    for gi in range(XB):
                        g = g0 + gi
                        ps = pp1.tile([128, 4 * H], mybir.dt.float32, space="PSUM",
                                      tag="h_ps", bufs=3)
                        for cc in range(4):
                            for a in range(2):
                                nc.tensor.matmul(
                                    out=ps[:, cc * H:(cc + 1) * H],
                                    lhsT=x_t[:, a, (gi * 4 + cc) * 128:(gi * 4 + cc + 1) * 128],
                                    rhs=(w0 if a == 0 else w1)[:],
                                    start=(a == 0), stop=(a == 1))
                        nc.scalar.activation(hb[:, gi, :], ps[:],
                                             mybir.ActivationFunctionType.Copy)
                        if g < NOWN:
                            nc.vector.tensor_copy(hs_own[:, g, :], ps[:])
                    # write XB groups; split at quarter boundaries
                    r_lo = g0 * GRP
                    while r_lo < (g0 + XB) * GRP:
                        qk = r_lo // QTAB
                        r_hi = min((g0 + XB) * GRP, (qk + 1) * QTAB)
                        gl = (r_lo - g0 * GRP) // GRP
                        gh = (r_hi - g0 * GRP) // GRP
                        dview = hs_tab[qk][r_lo - qk * QTAB:r_hi - qk * QTAB, :]
                        if gh - gl == 1:
                            nc.scalar.dma_start(
                                dview.rearrange("(p c) k -> p (c k)", p=128, c=4),
                                hb[:, gl, :])
                        else:
                            nc.scalar.dma_start(
                                dview.rearrange("(gi p c) k -> p gi (c k)", p=128, c=4),
                                hb[:, gl:gh, :])
                        r_lo = r_hi

            # ---------------- phase 3 ----------------
            with tc.tile_pool(name="psum3", bufs=WG, space="PSUM") as pp3:
                for gg in range(NG):
                    wlo = gg * WG
                    whi = min(wlo + WG, NW)
                    nwin = whi - wlo
                    pw = {}
                    for w in range(wlo, whi):
                        pwt = pp3.tile([128, H], mybir.dt.float32, space="PSUM",
                                       tag="pw", name=f"pw{w}", bufs=8)
                        pw[w] = pwt[:]
                        if self_mode == "sel":
                            # self-loop contribution from SBUF-resident own rows
                            r = w % 4
                            for c in range(4):
                                nc.tensor.matmul(
                                    out=pw[w], lhsT=selmat[r][c][:],
                                    rhs=hs_own[:, w // 4, c * 128:(c + 1) * 128],
                                    start=(c == 0), stop=False)

                    rd_sb = sb.tile([128, max(nblk_of_group)], mybir.dt.float32,
                                    tag="rd", bufs=3)
                    c0 = first_col_of_group[gg]
                    nc.sync.dma_start(rd_sb[:, 0:nblk_of_group[gg]],
                                      rel_d[:, c0:c0 + nblk_of_group[gg]])

                    for ci, (g_c, qq, off_idx, nidx, mms) in enumerate(calls):
                        if g_c != gg:
                            continue
                        g_t = sb.tile([128, max_call_blk, H], mybir.dt.bfloat16,
                                      tag="g_t", bufs=7)
                        nc.gpsimd.dma_gather(
                            g_t[:, 0:nidx // 128, :], hs_tab[qq][:, :],
                            idx_all[:, off_idx // 16:(off_idx + nidx) // 16],
                            nidx, nidx, H, single_packet=False)
                        b0 = off_idx // WIN
                        for (bcol, w, bank) in mms:
                            s_t = sb.tile([128, 128], mybir.dt.bfloat16, tag="s_t", bufs=16)
                            lc = bcol - first_col_of_group[gg]
                            nc.vector.tensor_scalar(
                                out=s_t[:], in0=iota_bank[bank][:],
                                scalar1=rd_sb[:, lc:lc + 1], scalar2=None,
                                op0=mybir.AluOpType.is_equal)
                            nc.tensor.matmul(out=pw[w], lhsT=s_t[:],
                                             rhs=g_t[:, bcol - b0, :],
                                             start=(self_mode != "sel"
                                                    and first_block.get(w) == (bcol, w)),
                                             stop=(last_block.get(w) == (bcol, w)))

                    # epilogue, batched partition-major output DMA per group
                    o_g = sb.tile([128, WG * H], mybir.dt.bfloat16, tag="o_g", bufs=2)
                    for w in range(wlo, whi):
                        og = o_g[:, (w - wlo) * H:(w - wlo + 1) * H]
                        if fast_epilogue:
                            # b == 0, uniform alpha<=1: out = max(dinv*psum,
                            # alpha*dinv*psum)
                            al = float(fast_epilogue["alpha"])
                            u = sb.tile([128, H], mybir.dt.float32, tag="u", bufs=4)
                            nc.scalar.activation(u[:], pw[w],
                                                 mybir.ActivationFunctionType.Copy,
                                                 scale=dinv_sb[:, w:w + 1])
                            t = sb.tile([128, H], mybir.dt.float32, tag="t", bufs=4)
                            nc.vector.tensor_scalar(
                                out=t[:], in0=pw[w],
                                scalar1=dinv_sb[:, w:w + 1], scalar2=al,
                                op0=mybir.AluOpType.mult, op1=mybir.AluOpType.mult)
                            nc.vector.tensor_tensor(out=og, in0=u[:], in1=t[:],
                                                    op=mybir.AluOpType.max)
                        else:
                            u = sb.tile([128, H], mybir.dt.float32, tag="u", bufs=4)
                            nc.scalar.activation(u[:], pw[w],
                                                 mybir.ActivationFunctionType.Copy,
                                                 scale=dinv_sb[:, w:w + 1])
                            u2 = sb.tile([128, H], mybir.dt.float32, tag="u2", bufs=4)
                            nc.vector.tensor_tensor(out=u2[:], in0=u[:], in1=b128[:],
                                                    op=mybir.AluOpType.add)
                            r2 = sb.tile([128, H], mybir.dt.float32, tag="r2", bufs=3)
                            nc.scalar.activation(r2[:], u2[:],
                                                 mybir.ActivationFunctionType.Relu,
                                                 scale=-1.0)
                            m = sb.tile([128, H], mybir.dt.float32, tag="m", bufs=3)
                            nc.gpsimd.tensor_tensor(out=m[:], in0=r2[:], in1=a128[:],
                                                    op=mybir.AluOpType.mult)
                            r1 = sb.tile([128, H], mybir.dt.float32, tag="r1", bufs=3)
                            nc.scalar.activation(r1[:], u2[:],
                                                 mybir.ActivationFunctionType.Relu)
                            nc.vector.tensor_tensor(out=og,
                                                    in0=r1[:], in1=m[:],
                                                    op=mybir.AluOpType.subtract)
                    if gg == NG - 1 and nwin >= 2:
                        h1 = nwin // 2
                        nc.scalar.dma_start(out_d[:, wlo * H:(wlo + h1) * H],
                                            o_g[:, 0:h1 * H])
                        nc.scalar.dma_start(out_d[:, (wlo + h1) * H:whi * H],
                                            o_g[:, h1 * H:nwin * H])
                    else:
                        nc.scalar.dma_start(
                            out_d[:, wlo * H:whi * H],
                            o_g[:, 0:nwin * H])

    nc.compile()
    return nc


_LAST = {}


def kernel(x, edge_index, W, b, alpha):
    from concourse.bass_utils import run_bass_kernel_spmd

    x = np.asarray(x, dtype=np.float32)
    W = np.asarray(W, dtype=np.float32)
    b = np.asarray(b, dtype=np.float32)
    alpha = np.asarray(alpha, dtype=np.float32)

    sched, per_core, dinv_np = _preprocess(edge_index)
    fast = None
    if np.all(b == 0.0) and np.all(alpha == alpha.flat[0]) and 0.0 <= alpha.flat[0] <= 1.0:
        fast = {"alpha": float(alpha.flat[0])}
    nc = _build(sched, fast_epilogue=fast)
    _LAST["nc"] = nc
    _LAST["sched"] = sched

    # x' = dinv * x, padded; per-core: table slot t holds node (t + c*SH) % NP;
    # K-major columns permuted so col (g*4+cc)*128 + p holds slot g*512+p*4+cc.
    x_pad = np.zeros((NP, KIN), np.float32)
    x_pad[:N] = dinv_np[:N, None] * x
    slot_perm = np.arange(NP).reshape(NGRP, 128, 4).transpose(0, 2, 1).reshape(-1)
    # slot_perm[(g*4+cc)*128 + p] = g*512 + p*4 + cc

    w_bf = W.astype(bf16)

    in_maps = []
    for c in range(NCORES):
        node_of_col = (slot_perm + c * SH) % NP
        xt_c = np.ascontiguousarray(x_pad[node_of_col].T.astype(bf16))  # [256, NP]
        in_maps.append({
            "xt_perm": xt_c,
            "w_bf": w_bf, "b_vec": b, "a_vec": alpha,
            "dinv_own": per_core[c]["dinv"],
            "idx16": per_core[c]["idx16"],
            "reldst": per_core[c]["reldst"],
        })

    res = run_bass_kernel_spmd(nc, in_maps, core_ids=list(range(NCORES)))
    # out_pm[d, w*H+f] -> rows w*128+d
    outs = []
    for c in range(NCORES):
        o = res.results[c]["out_pm"].astype(np.float32).reshape(128, NW, H).transpose(1, 0, 2)
        outs.append(o.reshape(SH, H))
    out = np.concatenate(outs, axis=0)
    return np.ascontiguousarray(out[:N])
